# revision 68
# baseline (speedup 1.0000x reference)
"""Dual-modality (opt/sar) multiplicative cross-attention — TRN2 Bass kernel.

Reference computation (per sample n, C=64, HW=64*64=4096):
  q_m = W_q^m x + b_q^m ; k_m = W_k^m x + b_k^m ; v_m = W_v^m x + b_v^m   (m in {opt,sar})
  att = softmax(q_o k_o^T) * softmax(q_s k_s^T)        (elementwise, (HW,HW))
  out = (att @ v_o) * (att @ v_s)                      -> (C,H,W) layout

Restructured for the hardware:
  A_m = exp(S_m - SHIFT)   (unnormalized, constant shift — safe: |S| <= ~62)
  sums_m[i] = sum_j A_m[i,j]  (free via ACT accum during the exp pass)
  P = A_o * A_s   ->  U_m = P @ v_m
  out = (U_o * U_s) / (sums_o * sums_s)^2     (denominators folded at the end)

Structure (cost-model-driven; the ACT engine exp stream ~146us busy is the
bottleneck, so everything else is scheduled around keeping it dense —
TimelineSim total 174.1us vs the 196.0us v1 baseline):
  - biases folded into the projection matmuls: host appends a ones-row to x
    and a bias-row to each weight (stationary [65,64] f32r; bias enters the
    fp32 PSUM accumulation exactly like the old DVE bias-add did).
  - exp issued as [128,1536]-wide activations (6 per 128-row q-block instead
    of 8 narrower ones: fewer fixed SBUF-access + read-accumulator costs).
  - PSUM (8 banks, buffers are bank-granular): S double-buffer 2x[128,1536]
    (6 banks) + 1 shared misc bank (projection staging / P-transpose groups /
    v staging) + 1 U bank. GPSIMD cannot access PSUM at all, so every
    PSUM->SBUF copy is on DVE, or on an ACT Copy-activation during the
    prologue while ACT is still idle.
  - q/k projections stage through the big S PSUM slots (no extra banks,
    bufs=2 ping-pong keeps PE from lock-stepping with the copies).
  - v produced directly in [j, c] layout (stationary x-chunk, moving W_v,
    64-col matmuls), staged through the misc bank during blocks 1-2 —
    removes the old vT + PE-transpose + copy pipeline entirely.
  - 4-deep software pipeline, pieces interleaved between individual S
    chunks so the in-order PE queue never head-of-line-blocks the next
    exp's matmuls:
      stage pre  (lag 1): P = A_o*A_s in quarters (2 DVE + 2 Pool),
                          sums -> 1/(sums_o*sums_s)^2
      stage tail (lag 2): P transpose groups + U accumulation, U trailing
                          the PT copies by one group
      stage out  (lag 3): U_o*U_s*inv2, transpose, store, DMA
  - single wall DMA for all six weight stacks; x/xq transfers sliced by
    first-need on the SP/ACT queues only (gpsimd DMA costs the Pool engine
    ~1us each); ~3us of dummy PE transposes warm the p-state ramp.

Sharding: 8 cores, core c handles sample c//2, query-row half c%2 (2048 rows).
Dtypes: S matmuls in float32r; A/P/v and the P@v matmuls in bf16;
sums/denominators in fp32.
"""
import numpy as np
from contextlib import ExitStack

import concourse.bass as bass
import concourse.tile as tile
from concourse import bacc, mybir
from concourse import masks
from concourse.bass_utils import run_bass_kernel_spmd

N, C, H, W = 4, 64, 64, 64
HW = H * W            # 4096
HALF = HW // 2        # 2048 query rows per core
NBLK = HALF // 128    # 16 q-blocks per core
SHIFT = 30.0
CHUNKS = [(0, 1536), (1536, 1536), (3072, 1024)]   # exp chunk (offset, width)

dt = mybir.dt
AF = mybir.ActivationFunctionType
ALU = mybir.AluOpType

_compiled = None


def _build(repeat=1):
    nc = bacc.Bacc("TRN2", debug=False)
    d_in = {}
    for m in ("opt", "sar"):
        d_in[f"xa_{m}"] = nc.dram_tensor(f"xa_{m}", (C + 1, HW), dt.float32r, kind="ExternalInput").ap()
        d_in[f"xqa_{m}"] = nc.dram_tensor(f"xqa_{m}", (C + 1, HALF), dt.float32r, kind="ExternalInput").ap()
    # all six weight+bias stacks side by side: [q_o|k_o|v_o|q_s|k_s|v_s]
    d_in["wall"] = nc.dram_tensor("wall", (C + 1, 6 * C), dt.float32r, kind="ExternalInput").ap()
    d_out = nc.dram_tensor("out", (C, HALF), dt.float32, kind="ExternalOutput").ap()

    with tile.TileContext(nc) as tc, ExitStack() as ctx:
        consts = ctx.enter_context(tc.tile_pool(name="consts", bufs=1))
        proj = ctx.enter_context(tc.tile_pool(name="proj", bufs=1))
        work = ctx.enter_context(tc.tile_pool(name="work", bufs=2))
        stats = ctx.enter_context(tc.tile_pool(name="stats", bufs=2))
        outp = ctx.enter_context(tc.tile_pool(name="outp", bufs=1))
        ps_S = ctx.enter_context(tc.tile_pool(name="ps_S", bufs=2, space="PSUM"))
        ps_T = ctx.enter_context(tc.tile_pool(name="ps_T", bufs=1, space="PSUM"))
        ps_U = ctx.enter_context(tc.tile_pool(name="ps_U", bufs=1, space="PSUM"))

        ident_bf = consts.tile([128, 128], dt.bfloat16)
        masks.make_identity(nc, ident_bf[:])
        ident_f32 = consts.tile([128, 128], dt.float32)
        masks.make_identity(nc, ident_f32[:])
        neg_shift = consts.tile([128, 1], dt.float32)
        nc.gpsimd.memset(neg_shift[:], -SHIFT)
        warm = consts.tile([128, 1], dt.float32)
        nc.scalar.activation(warm[:], neg_shift[:], AF.Exp)

        # PE p-state warm-up: ~3us of dummy transposes while the input DMA is
        # in flight, so the projection/S matmuls start at the full 2.4 GHz
        # clock instead of ramping through the slow p-states.
        for _wu in range(8):
            psw = ps_T.tile([128, 128], dt.bfloat16, tag="ps")
            nc.tensor.transpose(psw[:], ident_bf[:], ident_bf[:])

        # ---- input DMA: few transfers, first-needed first, SP+ACT queues
        # only (gpsimd DMA costs the Pool ENGINE ~1us per transfer) ----
        wall = consts.tile([C + 1, 6 * C], dt.float32r)
        nc.sync.dma_start(wall[:], d_in["wall"][:])
        wr = {}
        for mi, m in enumerate(("opt", "sar")):
            for pi, p in enumerate(("q", "k", "v")):
                wr[(p, m)] = wall[:, (3 * mi + pi) * C:(3 * mi + pi + 1) * C]

        xr_ctx = ExitStack()
        xrpool = xr_ctx.enter_context(tc.tile_pool(name="xr", bufs=1))
        x_r = {}
        xq_r = {}
        for m in ("opt", "sar"):
            xqr = xrpool.tile([C + 1, HALF], dt.float32r, tag=f"xqr_{m}")
            xq_r[m] = xqr
            xrt = xrpool.tile([C + 1, HW], dt.float32r, tag=f"xr_{m}")
            x_r[m] = xrt

        def _xq(m, lo, hi, eng):
            eng.dma_start(xq_r[m][:, lo:hi], d_in[f"xqa_{m}"][:, lo:hi])

        def _x(m, lo, hi, eng):
            eng.dma_start(x_r[m][:, lo:hi], d_in[f"xa_{m}"][:, lo:hi])

        _xq("opt", 0, 512, nc.sync)
        _xq("sar", 0, 512, nc.scalar)
        _x("opt", 0, 1536, nc.sync)
        _x("sar", 0, 1536, nc.scalar)
        _x("opt", 1536, HW, nc.sync)
        _x("sar", 1536, HW, nc.scalar)
        _xq("opt", 512, HALF, nc.sync)
        _xq("sar", 512, HALF, nc.scalar)

        # ---- SBUF state ----
        # kT_stack / qT_stack: rows 0:64 = opt, rows 64:128 = sar (float32r)
        kT = proj.tile([128, HW], dt.float32r)
        qT = proj.tile([128, HALF], dt.float32r)
        v_both = proj.tile([128, HW], dt.bfloat16)
        out_stage = outp.tile([C, HALF], dt.float32)
        _alt = [0]

        def _psum_copy(dst_ap, src_ap, can_use_act):
            # GPSIMD cannot touch PSUM; PSUM->SBUF copies go to DVE, or to an
            # ACT Copy-activation during the prologue while ACT is still idle.
            _alt[0] += 1
            if can_use_act and _alt[0] % 2 == 0:
                nc.scalar.activation(dst_ap, src_ap, AF.Copy)
            else:
                nc.vector.tensor_copy(dst_ap, src_ap)

        def proj_group(dst, p, m, mi, xsrc, lo, hi, can_use_act=True):
            rows = slice(64 * mi, 64 * mi + 64)
            ps = ps_S.tile([128, 1536], dt.float32, tag="S")
            for sub in range((hi - lo) // 512):
                nc.tensor.matmul(ps[0:C, bass.ts(sub, 512)], wr[(p, m)],
                                 xsrc[:, lo + sub * 512:lo + (sub + 1) * 512],
                                 start=True, stop=True)
            _psum_copy(dst[rows, lo:hi], ps[0:C, 0:hi - lo], can_use_act)

        def emit_v_piece(g):
            # v_both col group g (4 j-chunks of 128): each 128-col block is
            # [v_opt (64 c) | v_sar (64 c)], partitions = the j dimension:
            # direct out = x_chunk^T @ W_v — no vT stage, no PE transposes.
            V = ps_T.tile([128, 512], dt.float32, tag="ps")
            for t in range(4):
                j = g * 4 + t
                nc.tensor.matmul(V[:, t * 128:t * 128 + 64],
                                 x_r["opt"][:, bass.ts(j, 128)],
                                 wr[("v", "opt")], start=True, stop=True)
                nc.tensor.matmul(V[:, t * 128 + 64:t * 128 + 128],
                                 x_r["sar"][:, bass.ts(j, 128)],
                                 wr[("v", "sar")], start=True, stop=True)
            nc.vector.tensor_copy(v_both[:, bass.ts(g, 512)], V[:])

        # ---- pipeline stages ----
        def emit_S_chunk(d, ci, mi):
            i = d["i"]
            off, w = CHUNKS[ci]
            rows = slice(64 * mi, 64 * mi + 64)
            ps = ps_S.tile([128, 1536], dt.float32, tag="S")
            for sub in range(w // 512):
                nc.tensor.matmul(ps[:, bass.ts(sub, 512)], qT[rows, bass.ts(i, 128)],
                                 kT[rows, off + sub * 512:off + (sub + 1) * 512],
                                 tile_position=(64 * mi, 0), start=True, stop=True)
            m = ("opt", "sar")[mi]
            nc.scalar.activation(d["A"][m][:, off:off + w], ps[:, 0:w], AF.Exp,
                                 bias=neg_shift[:], accum_out=d["parts"][m][:, ci:ci + 1])

        def emit_pre_piece(d, k, dve=False):
            # P = A_o*A_s in quarters: chunks 0-1 on DVE (slotted between PT
            # copies), chunks 2-3 on Pool — keeps DVE free for the copies the
            # in-flight U accumulation is waiting on. In the drain (dve=True)
            # Pool's 3.6x-slower multiply would gate the final tail, so all
            # chunks go to DVE.
            if k == 0:
                P = work.tile([128, HW], dt.bfloat16, tag="P")
                d["P"] = P
            eng = nc.vector if (dve or k < 2) else nc.gpsimd
            sl = bass.ts(k, HW // 4)
            eng.tensor_mul(d["P"][:, sl], d["A"]["opt"][:, sl], d["A"]["sar"][:, sl])

        def emit_pre_fin(d):
            sums = {}
            for m in ("opt", "sar"):
                sm = stats.tile([128, 1], dt.float32, tag=f"sums_{m}", bufs=3)
                nc.vector.reduce_sum(sm[:], d["parts"][m][:], axis=mybir.AxisListType.X)
                sums[m] = sm
            denom = stats.tile([128, 1], dt.float32, tag="denom", bufs=3)
            nc.vector.tensor_mul(denom[:], sums["opt"][:], sums["sar"][:])
            inv = stats.tile([128, 1], dt.float32, tag="inv", bufs=3)
            nc.vector.reciprocal(inv[:], denom[:])
            inv2 = stats.tile([128, 1], dt.float32, tag="inv2", bufs=3)
            nc.vector.tensor_mul(inv2[:], inv[:], inv[:])
            d["inv2"] = inv2

        def emit_tail_piece(d, k, drain=False):
            # k in 0..3: transpose group k of P, then U j-chunks 8k..8k+7
            # of group k-1 (U accumulation trails the PT copies by one group).
            # During the final drain the S slots are free: stage through them
            # to break the single-buffer transpose->copy serial chain.
            if k == 0:
                PT = work.tile([128, HW], dt.bfloat16, tag="PT")
                d["PT"] = PT
                U = ps_U.tile([128, 128], dt.float32, tag="U")
                d["U"] = U
            if drain:
                pst = ps_S.tile([128, 1024], dt.bfloat16, tag="S")
            else:
                pst = ps_T.tile([128, 1024], dt.bfloat16, tag="ps")
            for t in range(8):
                j = k * 8 + t
                nc.tensor.transpose(pst[:, bass.ts(t, 128)],
                                    d["P"][:, bass.ts(j, 128)], ident_bf[:])
            # PT copies stay on DVE: they gate the U accumulation chain and
            # Pool's copy is 2.3x slower.
            nc.vector.tensor_copy(d["PT"][:, bass.ts(k, 1024)], pst[:])

        def emit_U_piece(d, k):
            U = d["U"]
            for j in range(8 * k, 8 * k + 8):
                nc.tensor.matmul(U[:], d["PT"][:, bass.ts(j, 128)],
                                 v_both[:, bass.ts(j, 128)],
                                 start=(j == 0), stop=(j == HW // 128 - 1))

        def emit_out(d):
            # epilogue on Pool (+ one PE transpose): DVE is reserved for the
            # PT-copy chain.
            i, U, inv2 = d["i"], d["U"], d["inv2"]
            t0 = stats.tile([128, 64], dt.float32, tag="t0")
            nc.vector.tensor_scalar_mul(t0[:], U[:, 0:64], inv2[:])
            ob = stats.tile([128, 64], dt.float32, tag="ob")
            nc.vector.tensor_mul(ob[:], t0[:], U[:, 64:128])
            nc.tensor.transpose(U[0:64, :], ob[:], ident_f32[:])
            nc.vector.tensor_copy(out_stage[:, bass.ts(i, 128)], U[0:C, :])
            nc.sync.dma_start(d_out[:, bass.ts(i, 128)], out_stage[:, bass.ts(i, 128)])

        # ---- emission driver: 4-deep pipeline, pieces between S chunks ----
        # hook order within a block (6 chunk slots):
        #   after c0s: pre(lag1), out(lag3), transp g0(lag2)
        #   after c1o: U g0, transp g1      after c1s: U g1, transp g2
        #   after c2o: U g2, transp g3      after c2s: U g3
        pipe = []

        def new_block(i):
            d = {"i": i, "A": {}, "parts": {}}
            for m in ("opt", "sar"):
                At = work.tile([128, HW], dt.bfloat16, tag=f"A_{m}", bufs=3)
                pt = stats.tile([128, len(CHUNKS)], dt.float32, tag=f"part_{m}", bufs=3)
                d["A"][m] = At
                d["parts"][m] = pt
            return d

        extra = []   # one-shot emissions (v pieces / leftover projections)

        def emit_block(i, startup=None):
            d = new_block(i)
            d1 = pipe[-1] if len(pipe) >= 1 else None   # lag 1
            d2 = pipe[-2] if len(pipe) >= 2 else None   # lag 2
            d3 = pipe[-3] if len(pipe) >= 3 else None   # lag 3
            slot = 0
            for ci in range(len(CHUNKS)):
                for mi in range(2):
                    if startup:
                        startup(ci, mi)
                    emit_S_chunk(d, ci, mi)
                    if slot == 1:
                        if d3 is not None:
                            emit_out(d3)
                        if d2 is not None:
                            emit_tail_piece(d2, 0)
                    elif slot >= 2:
                        if d2 is not None:
                            emit_U_piece(d2, slot - 2)
                            if slot <= 4:
                                emit_tail_piece(d2, slot - 1)
                        if d1 is not None:
                            if slot == 2:
                                emit_pre_piece(d1, 0)
                                emit_pre_piece(d1, 2)
                            elif slot == 3:
                                emit_pre_piece(d1, 1)
                            elif slot == 4:
                                emit_pre_piece(d1, 3)
                            else:
                                emit_pre_fin(d1)
                    for _ in range(2):
                        if extra:
                            extra.pop(0)()
                    slot += 1
            pipe.append(d)

        # prologue projections: narrow qT first so block 0 starts ASAP
        for m_i, m in enumerate(("opt", "sar")):
            proj_group(qT, "q", m, m_i, xq_r[m], 0, 512)

        def startup0(ci, mi):
            if mi == 0:
                off, w_ = CHUNKS[ci]
                for mj, mm in enumerate(("opt", "sar")):
                    proj_group(kT, "k", mm, mj, x_r[mm], off, off + w_)

        emit_block(0, startup=startup0)
        # remaining qT columns + v pieces fill blocks 1-2's piece slots
        for m_i, m in enumerate(("opt", "sar")):
            extra.append(lambda m_i=m_i, m=m: proj_group(qT, "q", m, m_i, xq_r[m],
                                                         512, 2048, can_use_act=False))
        for g in range(8):
            extra.append(lambda g=g: emit_v_piece(g))

        total = NBLK * repeat
        for b in range(1, total):
            emit_block(b % NBLK)
        while extra:
            extra.pop(0)()
        # drain the pipeline
        d1, d2, d3 = pipe[-1], pipe[-2], pipe[-3]
        emit_out(d3)
        # d2 tail first so its PT copies lead the DVE queue; transposes
        # alternate into the now-free S slots so group k+1 never waits
        # copy k; d1's pre runs on Pool/DVE behind the copies.
        emit_tail_piece(d2, 0)
        emit_tail_piece(d2, 1)
        emit_pre_piece(d1, 0)
        emit_U_piece(d2, 0)
        emit_tail_piece(d2, 2)
        emit_pre_piece(d1, 2)
        emit_U_piece(d2, 1)
        emit_tail_piece(d2, 3)
        emit_pre_piece(d1, 1)
        emit_U_piece(d2, 2)
        emit_pre_piece(d1, 3)
        emit_U_piece(d2, 3)
        emit_pre_fin(d1)
        emit_out(d2)
        emit_tail_piece(d1, 0)
        emit_tail_piece(d1, 1)
        emit_U_piece(d1, 0)
        emit_tail_piece(d1, 2)
        emit_U_piece(d1, 1)
        emit_tail_piece(d1, 3)
        emit_U_piece(d1, 2)
        emit_U_piece(d1, 3)
        emit_out(d1)
        xr_ctx.close()

    nc.compile()
    return nc


def _to_f32r(x):
    """Round fp32 to the float32r format: RNE to 11 mantissa bits, low 12 bits zero."""
    u = np.ascontiguousarray(x, np.float32).view(np.uint32)
    lsb = (u >> 12) & 1
    r = (u + np.uint32(0x7FF) + lsb) & np.uint32(0xFFFFF000)
    return r.view(np.float32)


def kernel(x_opt, x_sar, wq_opt, bq_opt, wk_opt, bk_opt, wv_opt, bv_opt,
           wq_sar, bq_sar, wk_sar, bk_sar, wv_sar, bv_sar, _trace=False):
    global _compiled
    if _compiled is None:
        _compiled = _build()
    nc = _compiled

    # weight+bias stacks (rows 0:64 = W^T, row 64 = bias), all six side by
    # side: [q_o|k_o|v_o|q_s|k_s|v_s] -> (65, 384) f32r
    stacks = []
    for m, wq, bq, wk, bk, wv, bv in (
            ("opt", wq_opt, bq_opt, wk_opt, bk_opt, wv_opt, bv_opt),
            ("sar", wq_sar, bq_sar, wk_sar, bk_sar, wv_sar, bv_sar)):
        for p, w_, b_ in (("q", wq, bq), ("k", wk, bk), ("v", wv, bv)):
            stacks.append(np.vstack([np.asarray(w_, np.float32).T,
                                     np.asarray(b_, np.float32)[None, :]]))
    ws = {"wall": _to_f32r(np.hstack(stacks))}

    in_maps = []
    for core in range(8):
        n, h = core // 2, core % 2
        m = dict(ws)
        ones = np.ones((1, HW), np.float32)
        for mod, x_ in (("opt", x_opt), ("sar", x_sar)):
            xa = _to_f32r(np.asarray(x_[n], np.float32).reshape(C, HW))
            xa = np.vstack([xa, ones])
            m[f"xa_{mod}"] = np.ascontiguousarray(xa)
            m[f"xqa_{mod}"] = np.ascontiguousarray(xa[:, h * HALF:(h + 1) * HALF])
        in_maps.append(m)

    kernel._last_in_maps = in_maps
    r = run_bass_kernel_spmd(nc, in_maps, core_ids=list(range(8)), trace=_trace)
    out = np.empty((N, C, HW), np.float32)
    for core in range(8):
        n, h = core // 2, core % 2
        out[n][:, h * HALF:(h + 1) * HALF] = r.results[core]["out"]
    kernel._last_result = r
    return out.reshape(N, C, H, W)


# revision 82
# speedup vs baseline: 1.0152x; 1.0152x over previous
"""Dual-modality (opt/sar) multiplicative cross-attention — TRN2 Bass kernel.

Reference computation (per sample n, C=64, HW=64*64=4096):
  q_m = W_q^m x + b_q^m ; k_m = W_k^m x + b_k^m ; v_m = W_v^m x + b_v^m   (m in {opt,sar})
  att = softmax(q_o k_o^T) * softmax(q_s k_s^T)        (elementwise, (HW,HW))
  out = (att @ v_o) * (att @ v_s)                      -> (C,H,W) layout

Restructured for the hardware:
  A_m = exp(S_m - SHIFT)   (unnormalized, constant shift — safe: |S| <= ~62)
  sums_m[i] = sum_j A_m[i,j]  (free via ACT accum during the exp pass)
  P = A_o * A_s   ->  U_m = P @ v_m
  out = (U_o * U_s) / (sums_o * sums_s)^2     (denominators folded at the end)

Structure (cost-model-driven; the ACT engine exp stream ~146us busy is the
bottleneck, so everything else is scheduled around keeping it dense —
TimelineSim total 171.5us vs the 196.0us v1 baseline):
  - biases folded into the projection matmuls: host appends a ones-row to x
    and a bias-row to each weight (stationary [65,64] f32r; bias enters the
    fp32 PSUM accumulation exactly like the old DVE bias-add did).
  - exp issued as [128,1536]-wide activations (6 per 128-row q-block instead
    of 8 narrower ones: fewer fixed SBUF-access + read-accumulator costs).
  - PSUM (8 banks, buffers are bank-granular): S double-buffer 2x[128,1536]
    (6 banks) + 1 shared misc bank (projection staging / P-transpose groups /
    v staging) + 1 U bank. GPSIMD cannot access PSUM at all, so every
    PSUM->SBUF copy is on DVE, or on an ACT Copy-activation during the
    prologue while ACT is still idle.
  - q/k projections stage through the big S PSUM slots (no extra banks,
    bufs=2 ping-pong keeps PE from lock-stepping with the copies).
  - v produced directly in [j, c] layout (stationary x-chunk, moving W_v,
    64-col matmuls), staged through the misc bank during blocks 1-2 —
    removes the old vT + PE-transpose + copy pipeline entirely.
  - 4-deep software pipeline, pieces interleaved between individual S
    chunks so the in-order PE queue never head-of-line-blocks the next
    exp's matmuls:
      stage pre  (lag 1): P = A_o*A_s in quarters (2 DVE + 2 Pool),
                          sums -> 1/(sums_o*sums_s)^2
      stage tail (lag 2): P transpose groups + U accumulation, U trailing
                          the PT copies by one group
      stage out  (lag 3): U_o*U_s*inv2, transpose, store, DMA
  - single wall DMA carries all six weight stacks PLUS the first 512 xq
    columns of each modality (the whole first projection's working set in
    one transfer — HWDGE fixed cost dominates small transfers); remaining
    x/xq sliced by first-need on the SP/ACT queues only (gpsimd DMA costs
    the Pool engine ~1us each); dummy PE transposes warm the p-state ramp.

Sharding: 8 cores, core c handles sample c//2, query-row half c%2 (2048 rows).
Dtypes: S matmuls in float32r; A/P/v and the P@v matmuls in bf16;
sums/denominators in fp32.
"""
import numpy as np
from contextlib import ExitStack

import concourse.bass as bass
import concourse.tile as tile
from concourse import bacc, mybir
from concourse import masks
from concourse.bass_utils import run_bass_kernel_spmd

N, C, H, W = 4, 64, 64, 64
HW = H * W            # 4096
HALF = HW // 2        # 2048 query rows per core
NBLK = HALF // 128    # 16 q-blocks per core
SHIFT = 30.0
CHUNKS = [(0, 1536), (1536, 1536), (3072, 1024)]   # exp chunk (offset, width)

dt = mybir.dt
AF = mybir.ActivationFunctionType
ALU = mybir.AluOpType

_compiled = None


def _build(repeat=1):
    nc = bacc.Bacc("TRN2", debug=False)
    d_in = {}
    for m in ("opt", "sar"):
        d_in[f"xa_{m}"] = nc.dram_tensor(f"xa_{m}", (C + 1, HW), dt.float32r, kind="ExternalInput").ap()
        d_in[f"xqa_{m}"] = nc.dram_tensor(f"xqa_{m}", (C + 1, HALF), dt.float32r, kind="ExternalInput").ap()
    # all six weight+bias stacks side by side: [q_o|k_o|v_o|q_s|k_s|v_s],
    # plus the first 512 xq columns of each modality so the whole first
    # projection's working set arrives in ONE transfer (HWDGE fixed cost).
    d_in["wall"] = nc.dram_tensor("wall", (C + 1, 6 * C + 1024), dt.float32r, kind="ExternalInput").ap()
    d_out = nc.dram_tensor("out", (C, HALF), dt.float32, kind="ExternalOutput").ap()

    with tile.TileContext(nc) as tc, ExitStack() as ctx:
        consts = ctx.enter_context(tc.tile_pool(name="consts", bufs=1))
        proj = ctx.enter_context(tc.tile_pool(name="proj", bufs=1))
        work = ctx.enter_context(tc.tile_pool(name="work", bufs=2))
        stats = ctx.enter_context(tc.tile_pool(name="stats", bufs=2))
        outp = ctx.enter_context(tc.tile_pool(name="outp", bufs=1))
        ps_S = ctx.enter_context(tc.tile_pool(name="ps_S", bufs=2, space="PSUM"))
        ps_T = ctx.enter_context(tc.tile_pool(name="ps_T", bufs=1, space="PSUM"))
        ps_U = ctx.enter_context(tc.tile_pool(name="ps_U", bufs=1, space="PSUM"))

        ident_bf = consts.tile([128, 128], dt.bfloat16)
        masks.make_identity(nc, ident_bf[:])
        ident_f32 = consts.tile([128, 128], dt.float32)
        masks.make_identity(nc, ident_f32[:])
        neg_shift = consts.tile([128, 1], dt.float32)
        nc.gpsimd.memset(neg_shift[:], -SHIFT)
        warm = consts.tile([128, 1], dt.float32)
        nc.scalar.activation(warm[:], neg_shift[:], AF.Exp)

        # PE p-state warm-up: ~3us of dummy transposes while the input DMA is
        # in flight, so the projection/S matmuls start at the full 2.4 GHz
        # clock instead of ramping through the slow p-states.
        for _wu in range(8):
            psw = ps_T.tile([128, 128], dt.bfloat16, tag="ps")
            nc.tensor.transpose(psw[:], ident_bf[:], ident_bf[:])

        # ---- input DMA: few transfers, first-needed first, SP+ACT queues
        # only (gpsimd DMA costs the Pool ENGINE ~1us per transfer) ----
        wall = consts.tile([C + 1, 6 * C + 1024], dt.float32r)
        nc.sync.dma_start(wall[:], d_in["wall"][:])
        wr = {}
        for mi, m in enumerate(("opt", "sar")):
            for pi, p in enumerate(("q", "k", "v")):
                wr[(p, m)] = wall[:, (3 * mi + pi) * C:(3 * mi + pi + 1) * C]

        xr_ctx = ExitStack()
        xrpool = xr_ctx.enter_context(tc.tile_pool(name="xr", bufs=1))
        x_r = {}
        xq_r = {}
        for m in ("opt", "sar"):
            xqr = xrpool.tile([C + 1, HALF], dt.float32r, tag=f"xqr_{m}")
            xq_r[m] = xqr
            xrt = xrpool.tile([C + 1, HW], dt.float32r, tag=f"xr_{m}")
            x_r[m] = xrt

        def _xq(m, lo, hi, eng):
            eng.dma_start(xq_r[m][:, lo:hi], d_in[f"xqa_{m}"][:, lo:hi])

        def _x(m, lo, hi, eng):
            eng.dma_start(x_r[m][:, lo:hi], d_in[f"xa_{m}"][:, lo:hi])

        _x("opt", 0, 1536, nc.sync)
        _x("sar", 0, 1536, nc.scalar)
        _x("opt", 1536, HW, nc.sync)
        _x("sar", 1536, HW, nc.scalar)
        _xq("opt", 512, HALF, nc.sync)
        _xq("sar", 512, HALF, nc.scalar)

        # ---- SBUF state ----
        # kT_stack / qT_stack: rows 0:64 = opt, rows 64:128 = sar (float32r)
        kT = proj.tile([128, HW], dt.float32r)
        qT = proj.tile([128, HALF], dt.float32r)
        v_both = proj.tile([128, HW], dt.bfloat16)
        out_stage = outp.tile([C, HALF], dt.float32)
        _alt = [0]

        def _psum_copy(dst_ap, src_ap, can_use_act):
            # GPSIMD cannot touch PSUM; PSUM->SBUF copies go to DVE, or to an
            # ACT Copy-activation during the prologue while ACT is still idle.
            _alt[0] += 1
            if can_use_act and _alt[0] % 2 == 0:
                nc.scalar.activation(dst_ap, src_ap, AF.Copy)
            else:
                nc.vector.tensor_copy(dst_ap, src_ap)

        def proj_group(dst, p, m, mi, xsrc, lo, hi, can_use_act=True):
            rows = slice(64 * mi, 64 * mi + 64)
            ps = ps_S.tile([128, 1536], dt.float32, tag="S")
            for sub in range((hi - lo) // 512):
                nc.tensor.matmul(ps[0:C, bass.ts(sub, 512)], wr[(p, m)],
                                 xsrc[:, lo + sub * 512:lo + (sub + 1) * 512],
                                 start=True, stop=True)
            _psum_copy(dst[rows, lo:hi], ps[0:C, 0:hi - lo], can_use_act)

        def emit_v_piece(g):
            # v_both col group g (4 j-chunks of 128): each 128-col block is
            # [v_opt (64 c) | v_sar (64 c)], partitions = the j dimension:
            # direct out = x_chunk^T @ W_v — no vT stage, no PE transposes.
            V = ps_T.tile([128, 512], dt.float32, tag="ps")
            for t in range(4):
                j = g * 4 + t
                nc.tensor.matmul(V[:, t * 128:t * 128 + 64],
                                 x_r["opt"][:, bass.ts(j, 128)],
                                 wr[("v", "opt")], start=True, stop=True)
                nc.tensor.matmul(V[:, t * 128 + 64:t * 128 + 128],
                                 x_r["sar"][:, bass.ts(j, 128)],
                                 wr[("v", "sar")], start=True, stop=True)
            nc.vector.tensor_copy(v_both[:, bass.ts(g, 512)], V[:])

        # ---- pipeline stages ----
        def emit_S_chunk(d, ci, mi):
            i = d["i"]
            off, w = CHUNKS[ci]
            rows = slice(64 * mi, 64 * mi + 64)
            ps = ps_S.tile([128, 1536], dt.float32, tag="S")
            for sub in range(w // 512):
                nc.tensor.matmul(ps[:, bass.ts(sub, 512)], qT[rows, bass.ts(i, 128)],
                                 kT[rows, off + sub * 512:off + (sub + 1) * 512],
                                 tile_position=(64 * mi, 0), start=True, stop=True)
            m = ("opt", "sar")[mi]
            nc.scalar.activation(d["A"][m][:, off:off + w], ps[:, 0:w], AF.Exp,
                                 bias=neg_shift[:], accum_out=d["parts"][m][:, ci:ci + 1])

        def emit_pre_piece(d, k, dve=False):
            # P = A_o*A_s in quarters: chunks 0-1 on DVE (slotted between PT
            # copies), chunks 2-3 on Pool — keeps DVE free for the copies the
            # in-flight U accumulation is waiting on. In the drain (dve=True)
            # Pool's 3.6x-slower multiply would gate the final tail, so all
            # chunks go to DVE.
            if k == 0:
                P = work.tile([128, HW], dt.bfloat16, tag="P")
                d["P"] = P
            eng = nc.vector if (dve or k < 2) else nc.gpsimd
            sl = bass.ts(k, HW // 4)
            eng.tensor_mul(d["P"][:, sl], d["A"]["opt"][:, sl], d["A"]["sar"][:, sl])

        def emit_pre_fin(d):
            sums = {}
            for m in ("opt", "sar"):
                sm = stats.tile([128, 1], dt.float32, tag=f"sums_{m}", bufs=3)
                nc.vector.reduce_sum(sm[:], d["parts"][m][:], axis=mybir.AxisListType.X)
                sums[m] = sm
            denom = stats.tile([128, 1], dt.float32, tag="denom", bufs=3)
            nc.vector.tensor_mul(denom[:], sums["opt"][:], sums["sar"][:])
            inv = stats.tile([128, 1], dt.float32, tag="inv", bufs=3)
            nc.vector.reciprocal(inv[:], denom[:])
            inv2 = stats.tile([128, 1], dt.float32, tag="inv2", bufs=3)
            nc.vector.tensor_mul(inv2[:], inv[:], inv[:])
            d["inv2"] = inv2

        def emit_tail_piece(d, k, drain=False):
            # k in 0..3: transpose group k of P, then U j-chunks 8k..8k+7
            # of group k-1 (U accumulation trails the PT copies by one group).
            # During the final drain the S slots are free: stage through them
            # to break the single-buffer transpose->copy serial chain.
            if k == 0:
                PT = work.tile([128, HW], dt.bfloat16, tag="PT")
                d["PT"] = PT
                U = ps_U.tile([128, 128], dt.float32, tag="U")
                d["U"] = U
            if drain:
                pst = ps_S.tile([128, 1024], dt.bfloat16, tag="S")
            else:
                pst = ps_T.tile([128, 1024], dt.bfloat16, tag="ps")
            for t in range(8):
                j = k * 8 + t
                nc.tensor.transpose(pst[:, bass.ts(t, 128)],
                                    d["P"][:, bass.ts(j, 128)], ident_bf[:])
            # PT copies stay on DVE: they gate the U accumulation chain and
            # Pool's copy is 2.3x slower.
            nc.vector.tensor_copy(d["PT"][:, bass.ts(k, 1024)], pst[:])

        def emit_U_piece(d, k):
            U = d["U"]
            for j in range(8 * k, 8 * k + 8):
                nc.tensor.matmul(U[:], d["PT"][:, bass.ts(j, 128)],
                                 v_both[:, bass.ts(j, 128)],
                                 start=(j == 0), stop=(j == HW // 128 - 1))

        def emit_out(d):
            # epilogue on Pool (+ one PE transpose): DVE is reserved for the
            # PT-copy chain.
            i, U, inv2 = d["i"], d["U"], d["inv2"]
            t0 = stats.tile([128, 64], dt.float32, tag="t0")
            nc.vector.tensor_scalar_mul(t0[:], U[:, 0:64], inv2[:])
            ob = stats.tile([128, 64], dt.float32, tag="ob")
            nc.vector.tensor_mul(ob[:], t0[:], U[:, 64:128])
            nc.tensor.transpose(U[0:64, :], ob[:], ident_f32[:])
            nc.vector.tensor_copy(out_stage[:, bass.ts(i, 128)], U[0:C, :])
            nc.sync.dma_start(d_out[:, bass.ts(i, 128)], out_stage[:, bass.ts(i, 128)])

        # ---- emission driver: 4-deep pipeline, pieces between S chunks ----
        # hook order within a block (6 chunk slots):
        #   after c0s: pre(lag1), out(lag3), transp g0(lag2)
        #   after c1o: U g0, transp g1      after c1s: U g1, transp g2
        #   after c2o: U g2, transp g3      after c2s: U g3
        pipe = []

        def new_block(i):
            d = {"i": i, "A": {}, "parts": {}}
            for m in ("opt", "sar"):
                At = work.tile([128, HW], dt.bfloat16, tag=f"A_{m}", bufs=3)
                pt = stats.tile([128, len(CHUNKS)], dt.float32, tag=f"part_{m}", bufs=3)
                d["A"][m] = At
                d["parts"][m] = pt
            return d

        extra = []   # one-shot emissions (v pieces / leftover projections)

        def emit_block(i, startup=None):
            d = new_block(i)
            d1 = pipe[-1] if len(pipe) >= 1 else None   # lag 1
            d2 = pipe[-2] if len(pipe) >= 2 else None   # lag 2
            d3 = pipe[-3] if len(pipe) >= 3 else None   # lag 3
            slot = 0
            for ci in range(len(CHUNKS)):
                for mi in range(2):
                    if startup:
                        startup(ci, mi)
                    emit_S_chunk(d, ci, mi)
                    if slot == 1:
                        if d3 is not None:
                            emit_out(d3)
                        if d2 is not None:
                            emit_tail_piece(d2, 0)
                    elif slot >= 2:
                        if d2 is not None:
                            emit_U_piece(d2, slot - 2)
                            if slot <= 4:
                                emit_tail_piece(d2, slot - 1)
                        if d1 is not None:
                            if slot == 2:
                                emit_pre_piece(d1, 0)
                                emit_pre_piece(d1, 2)
                            elif slot == 3:
                                emit_pre_piece(d1, 1)
                            elif slot == 4:
                                emit_pre_piece(d1, 3)
                            else:
                                emit_pre_fin(d1)
                    for _ in range(2):
                        if extra:
                            extra.pop(0)()
                    slot += 1
            pipe.append(d)

        # prologue projections: narrow qT first so block 0 starts ASAP;
        # the first 512 xq columns ride inside the wall transfer.
        for m_i, m in enumerate(("opt", "sar")):
            rows = slice(64 * m_i, 64 * m_i + 64)
            ps = ps_S.tile([128, 1536], dt.float32, tag="S")
            nc.tensor.matmul(ps[0:C, 0:512], wr[("q", m)],
                             wall[:, 6 * C + 512 * m_i:6 * C + 512 * (m_i + 1)],
                             start=True, stop=True)
            _psum_copy(qT[rows, 0:512], ps[0:C, 0:512], True)

        def startup0(ci, mi):
            if mi == 0:
                off, w_ = CHUNKS[ci]
                for mj, mm in enumerate(("opt", "sar")):
                    proj_group(kT, "k", mm, mj, x_r[mm], off, off + w_)

        emit_block(0, startup=startup0)
        # remaining qT columns + v pieces fill blocks 1-2's piece slots
        for m_i, m in enumerate(("opt", "sar")):
            extra.append(lambda m_i=m_i, m=m: proj_group(qT, "q", m, m_i, xq_r[m],
                                                         512, 2048, can_use_act=False))
        for g in range(8):
            extra.append(lambda g=g: emit_v_piece(g))

        total = NBLK * repeat
        for b in range(1, total):
            emit_block(b % NBLK)
        while extra:
            extra.pop(0)()
        # drain the pipeline
        d1, d2, d3 = pipe[-1], pipe[-2], pipe[-3]
        emit_out(d3)
        # d2 tail first so its PT copies lead the DVE queue; transposes
        # alternate into the now-free S slots so group k+1 never waits
        # copy k; d1's pre runs on Pool/DVE behind the copies.
        emit_tail_piece(d2, 0)
        emit_tail_piece(d2, 1)
        emit_pre_piece(d1, 0)
        emit_U_piece(d2, 0)
        emit_tail_piece(d2, 2)
        emit_pre_piece(d1, 2)
        emit_U_piece(d2, 1)
        emit_tail_piece(d2, 3)
        emit_pre_piece(d1, 1)
        emit_U_piece(d2, 2)
        emit_pre_piece(d1, 3)
        emit_U_piece(d2, 3)
        emit_pre_fin(d1)
        emit_out(d2)
        emit_tail_piece(d1, 0)
        emit_tail_piece(d1, 1)
        emit_U_piece(d1, 0)
        emit_tail_piece(d1, 2)
        emit_U_piece(d1, 1)
        emit_tail_piece(d1, 3)
        emit_U_piece(d1, 2)
        emit_U_piece(d1, 3)
        emit_out(d1)
        xr_ctx.close()

    nc.compile()
    return nc


def _to_f32r(x):
    """Round fp32 to the float32r format: RNE to 11 mantissa bits, low 12 bits zero."""
    u = np.ascontiguousarray(x, np.float32).view(np.uint32)
    lsb = (u >> 12) & 1
    r = (u + np.uint32(0x7FF) + lsb) & np.uint32(0xFFFFF000)
    return r.view(np.float32)


def kernel(x_opt, x_sar, wq_opt, bq_opt, wk_opt, bk_opt, wv_opt, bv_opt,
           wq_sar, bq_sar, wk_sar, bk_sar, wv_sar, bv_sar, _trace=False):
    global _compiled
    if _compiled is None:
        _compiled = _build()
    nc = _compiled

    # weight+bias stacks (rows 0:64 = W^T, row 64 = bias), all six side by
    # side: [q_o|k_o|v_o|q_s|k_s|v_s] -> (65, 384) f32r
    stacks = []
    for m, wq, bq, wk, bk, wv, bv in (
            ("opt", wq_opt, bq_opt, wk_opt, bk_opt, wv_opt, bv_opt),
            ("sar", wq_sar, bq_sar, wk_sar, bk_sar, wv_sar, bv_sar)):
        for p, w_, b_ in (("q", wq, bq), ("k", wk, bk), ("v", wv, bv)):
            stacks.append(np.vstack([np.asarray(w_, np.float32).T,
                                     np.asarray(b_, np.float32)[None, :]]))
    wall_w = _to_f32r(np.hstack(stacks))

    in_maps = []
    for core in range(8):
        n, h = core // 2, core % 2
        m = {}
        ones = np.ones((1, HW), np.float32)
        for mod, x_ in (("opt", x_opt), ("sar", x_sar)):
            xa = _to_f32r(np.asarray(x_[n], np.float32).reshape(C, HW))
            xa = np.vstack([xa, ones])
            m[f"xa_{mod}"] = np.ascontiguousarray(xa)
            m[f"xqa_{mod}"] = np.ascontiguousarray(xa[:, h * HALF:(h + 1) * HALF])
        m["wall"] = np.ascontiguousarray(np.hstack(
            [wall_w, m["xqa_opt"][:, 0:512], m["xqa_sar"][:, 0:512]]))
        in_maps.append(m)

    kernel._last_in_maps = in_maps
    r = run_bass_kernel_spmd(nc, in_maps, core_ids=list(range(8)), trace=_trace)
    out = np.empty((N, C, HW), np.float32)
    for core in range(8):
        n, h = core // 2, core % 2
        out[n][:, h * HALF:(h + 1) * HALF] = r.results[core]["out"]
    kernel._last_result = r
    return out.reshape(N, C, H, W)
